# revision 5
# baseline (speedup 1.0000x reference)
"""Pixel multi-class InfoNCE loss on 8 TRN2 NeuronCores.

Reference computation (see problem):
  e = normalize(emb.transpose(0,2,3,1).reshape(N, C))       # N=8192, C=128
  logits = (e @ e.T) / 0.1, diag masked to -inf
  row_loss = -log(pos_sum / (tot_sum + eps))                 # per row
  loss = mean over present classes of per-class mean row_loss

Sharding: each of 8 cores owns 1024 rows (i) and computes its
[8192(j), 1024(i)] block of exp(logits^T) against the full embedding,
reducing per-class sums on the fly via a one-hot matmul.  Per-core
output is the per-class partial sum of row losses [4]; the host sums
partials and divides by class counts.
"""

import math

import ml_dtypes
import numpy as np
from contextlib import ExitStack

import concourse.bass as bass
import concourse.tile as tile
from concourse import bacc, mybir
from concourse.bass_utils import run_bass_kernel_spmd

F32 = mybir.dt.float32
BF16 = mybir.dt.bfloat16

N = 8192
C = 128
NCORES = 8
RPC = N // NCORES  # 1024 rows (i) per core
NCLS = 4
EPS = 1e-6
# rsqrt with the 1/temperature fold: s = sqrt(10/x) = exp(-0.5*ln(x) + 0.5*ln(10))
RSQRT_BIAS = 0.5 * math.log(10.0)

_NC_CACHE = []


def _build_body(tc, et, rows, ohj, ohi, cls):
    nc = tc.nc
    ctx = ExitStack()
    with ctx:
        pc = ctx.enter_context(tc.tile_pool(name="const", bufs=1))
        pbig = ctx.enter_context(tc.tile_pool(name="big", bufs=1))
        pex = ctx.enter_context(tc.tile_pool(name="ex", bufs=3))
        psm = ctx.enter_context(tc.tile_pool(name="sm", bufs=2))

        ones_bf = pc.tile([128, 1], BF16)
        nc.vector.memset(ones_bf[:], 1.0)
        ones_f = pc.tile([128, 1], F32)
        nc.vector.memset(ones_f[:], 1.0)
        ones_row = pc.tile([1, 128], F32)
        nc.vector.memset(ones_row[:], 1.0)
        bias_t = pc.tile([128, 1], F32)
        nc.vector.memset(bias_t[:], RSQRT_BIAS)

        ohj_sb = pc.tile([128, NCLS * (N // 128)], BF16)
        nc.sync.dma_start(ohj_sb[:], ohj[:])
        ohi_sb = pc.tile([NCLS, RPC], F32)
        nc.sync.dma_start(ohi_sb[:], ohi[:])

        eb = pc.tile([C, N], BF16)        # bf16 of raw E^T (all columns)
        sT = pc.tile([128, N // 128], F32)  # per-j scale, transposed layout
        erows = pc.tile([C, RPC], BF16)   # normalized rows slice (rhs)
        dexp = pc.tile([1, RPC], F32)     # exp of diag logit per local row
        cls_parts = pc.tile([NCLS, 2], F32)

        # ---------------- setup: rows side ----------------
        with tc.tile_pool(name="pssetup", bufs=2, space="PSUM") as pss:
            rows_raw = pbig.tile([C, RPC], F32)
            nc.sync.dma_start(rows_raw[:], rows[:])
            ebr = pbig.tile([C, RPC], BF16)
            nc.vector.tensor_copy(ebr[:], rows_raw[:])
            sqr = pbig.tile([C, RPC], BF16)
            nc.vector.tensor_mul(sqr[:], ebr[:], ebr[:])

            ssq_row = pbig.tile([1, RPC], F32)
            for t in range(RPC // 512):
                ps = pss.tile([1, 512], F32, tag="psrow")
                nc.tensor.matmul(ps[:], ones_bf[:], sqr[:, t * 512:(t + 1) * 512],
                                 start=True, stop=True)
                nc.vector.tensor_copy(ssq_row[:, t * 512:(t + 1) * 512], ps[:])
            s_row = pbig.tile([1, RPC], F32)
            nc.scalar.activation(s_row[:], ssq_row[:], mybir.ActivationFunctionType.Ln)
            nc.scalar.activation(s_row[:], s_row[:], mybir.ActivationFunctionType.Exp,
                                 scale=-0.5, bias=bias_t[0:1])

            # broadcast s_row across partitions via K=1 matmul, normalize rows
            for t in range(RPC // 512):
                sl = slice(t * 512, (t + 1) * 512)
                psb = pss.tile([128, 512], F32, tag="psb")
                nc.tensor.matmul(psb[:], ones_row[:], s_row[:, sl], start=True, stop=True)
                nc.vector.tensor_mul(erows[:, sl], rows_raw[:, sl], psb[:])

            # diag logit: sum_c b(raw)*b(e') * s_i, through the same rounding
            mix = pbig.tile([C, RPC], F32)
            nc.vector.tensor_mul(mix[:], ebr[:], erows[:])
            dv = pbig.tile([1, RPC], F32)
            for t in range(RPC // 512):
                sl = slice(t * 512, (t + 1) * 512)
                pd = pss.tile([1, 512], F32, tag="pd")
                nc.tensor.matmul(pd[:], ones_f[:], mix[:, sl], start=True, stop=True)
                nc.vector.tensor_copy(dv[:, sl], pd[:])
            dvs = pbig.tile([1, RPC], F32)
            nc.vector.tensor_mul(dvs[:], dv[:], s_row[:])
            dexp_b = pbig.tile([1, RPC], BF16)
            nc.scalar.activation(dexp_b[:], dvs[:], mybir.ActivationFunctionType.Exp)
            nc.vector.tensor_copy(dexp[:], dexp_b[:])

            # ---------------- setup: all-columns side ----------------
            et_raw = pbig.tile([C, N], F32)
            sqb = pbig.tile([C, N], BF16)
            CK = 1024
            for k in range(N // CK):
                sl = slice(k * CK, (k + 1) * CK)
                nc.sync.dma_start(et_raw[:, sl], et[:, sl])
                nc.vector.tensor_copy(eb[:, sl], et_raw[:, sl])
                nc.vector.tensor_mul(sqb[:, sl], eb[:, sl], eb[:, sl])

            ps_ssq = pss.tile([128, N // 128], F32, tag="psq")
            for jb in range(N // 128):
                nc.tensor.matmul(ps_ssq[:, jb:jb + 1],
                                 sqb[:, jb * 128:(jb + 1) * 128], ones_bf[:],
                                 start=True, stop=True)
            ssqT = pbig.tile([128, N // 128], F32)
            nc.vector.tensor_copy(ssqT[:], ps_ssq[:])
            nc.scalar.activation(sT[:], ssqT[:], mybir.ActivationFunctionType.Ln)
            nc.scalar.activation(sT[:], sT[:], mybir.ActivationFunctionType.Exp,
                                 scale=-0.5, bias=bias_t[:])

        # ---------------- main loop ----------------
        ssb = psm.tile([NCLS, RPC], F32, tag="ssb")
        with tc.tile_pool(name="pslg", bufs=2, space="PSUM") as pslg, \
             tc.tile_pool(name="psacc", bufs=1, space="PSUM") as psacc:
            s_acc = psacc.tile([NCLS, RPC], F32)
            nj = N // 128
            for jb in range(nj):
                lg = pslg.tile([128, RPC], F32, tag="lg")
                exb = pex.tile([128, RPC], BF16, tag="exb")
                for it in range(RPC // 512):
                    sl = slice(it * 512, (it + 1) * 512)
                    nc.tensor.matmul(lg[:, sl], eb[:, jb * 128:(jb + 1) * 128],
                                     erows[:, sl], start=True, stop=True)
                nc.scalar.activation(exb[:], lg[:], mybir.ActivationFunctionType.Exp,
                                     scale=sT[:, jb:jb + 1])
                for it in range(RPC // 512):
                    sl = slice(it * 512, (it + 1) * 512)
                    nc.tensor.matmul(s_acc[:, sl], ohj_sb[:, jb * NCLS:(jb + 1) * NCLS],
                                     exb[:, sl], start=(jb == 0), stop=(jb == nj - 1),
                                     skip_group_check=True)
            nc.vector.tensor_copy(ssb[:], s_acc[:])

        # ---------------- tail ----------------
        with tc.tile_pool(name="pstail", bufs=2, space="PSUM") as pstail:
            for it in range(RPC // 512):
                sl = slice(it * 512, (it + 1) * 512)
                pm = psm.tile([NCLS, 512], F32, tag="pm")
                nc.vector.tensor_mul(pm[:], ssb[:, sl], ohi_sb[:, sl])
                # sum the 4 class rows via K=4 ones-matmul (DVE cannot read
                # partition offsets 1..3)
                tot_ps = pstail.tile([1, 512], F32, tag="totps")
                nc.tensor.matmul(tot_ps[:], ones_f[0:NCLS, 0:1], ssb[:, sl],
                                 start=True, stop=True)
                pos_ps = pstail.tile([1, 512], F32, tag="posps")
                nc.tensor.matmul(pos_ps[:], ones_f[0:NCLS, 0:1], pm[:],
                                 start=True, stop=True)
                # remove diag term, add eps to denominator
                pos = psm.tile([1, 512], F32, tag="pos")
                tot = psm.tile([1, 512], F32, tag="tot")
                nc.vector.tensor_sub(pos[:], pos_ps[:], dexp[:, sl])
                nc.vector.tensor_sub(tot[:], tot_ps[:], dexp[:, sl])
                nc.vector.tensor_scalar_add(tot[:], tot[:], EPS)
                lt = psm.tile([1, 512], F32, tag="lt")
                lp = psm.tile([1, 512], F32, tag="lp")
                nc.scalar.activation(lt[:], tot[:], mybir.ActivationFunctionType.Ln)
                nc.scalar.activation(lp[:], pos[:], mybir.ActivationFunctionType.Ln)
                rl = psm.tile([1, 512], F32, tag="rl")
                nc.vector.tensor_sub(rl[:], lt[:], lp[:])
                # broadcast row_loss to 4 partitions, mask per class, reduce over i
                p4 = pstail.tile([NCLS, 512], F32, tag="p4")
                nc.tensor.matmul(p4[:], ones_row[0:1, 0:NCLS], rl[:], start=True, stop=True)
                cm = psm.tile([NCLS, 512], F32, tag="cm")
                nc.vector.tensor_mul(cm[:], p4[:], ohi_sb[:, sl])
                nc.vector.reduce_sum(cls_parts[:, it:it + 1], cm[:],
                                     axis=mybir.AxisListType.X)
            cls_f = psm.tile([NCLS, 1], F32, tag="clsf")
            nc.vector.tensor_add(cls_f[:], cls_parts[:, 0:1], cls_parts[:, 1:2])
            nc.sync.dma_start(cls[:], cls_f[:])


def build():
    nc = bacc.Bacc("TRN2", target_bir_lowering=False, debug=False)
    et = nc.dram_tensor("et", [C, N], F32, kind="ExternalInput").ap()
    rows = nc.dram_tensor("rows", [C, RPC], F32, kind="ExternalInput").ap()
    ohj = nc.dram_tensor("ohj", [128, NCLS * (N // 128)], BF16, kind="ExternalInput").ap()
    ohi = nc.dram_tensor("ohi", [NCLS, RPC], F32, kind="ExternalInput").ap()
    cls = nc.dram_tensor("cls", [NCLS, 1], F32, kind="ExternalOutput").ap()
    with tile.TileContext(nc) as tc:
        _build_body(tc, et, rows, ohj, ohi, cls)
    nc.compile()
    return nc


def _get_nc():
    if not _NC_CACHE:
        _NC_CACHE.append(build())
    return _NC_CACHE[0]


def make_in_maps(emb, labels):
    emb = np.asarray(emb, dtype=np.float32)
    lab = np.asarray(labels).astype(np.int64).reshape(-1)
    assert emb.shape == (2, C, 64, 64) and lab.shape == (N,)
    et = emb.reshape(2, C, N // 2)
    et = np.ascontiguousarray(np.concatenate([et[0], et[1]], axis=1))  # [C, N]
    oh = (lab[:, None] == np.arange(NCLS)[None, :]).astype(np.float32)  # [N, 4]
    ohj = np.ascontiguousarray(
        oh.reshape(N // 128, 128, NCLS).transpose(1, 0, 2).reshape(128, -1)
    ).astype(ml_dtypes.bfloat16)
    in_maps = []
    for r in range(NCORES):
        sl = slice(r * RPC, (r + 1) * RPC)
        in_maps.append({
            "et": et,
            "rows": np.ascontiguousarray(et[:, sl]),
            "ohj": ohj,
            "ohi": np.ascontiguousarray(oh[sl].T),
        })
    return in_maps, lab


def finish(results, lab):
    sums = np.zeros(NCLS, np.float64)
    for r in results:
        sums += r["cls"].reshape(NCLS).astype(np.float64)
    counts = np.bincount(lab, minlength=NCLS).astype(np.float64)
    present = counts > 0
    per_cls = np.where(present, sums / np.maximum(counts, 1.0), 0.0)
    return np.float32(per_cls.sum() / present.sum())


def kernel(emb, labels):
    in_maps, lab = make_in_maps(emb, labels)
    nc = _get_nc()
    res = run_bass_kernel_spmd(nc, in_maps, core_ids=list(range(NCORES)))
    return finish(res.results, lab)


# revision 7
# speedup vs baseline: 1.0361x; 1.0361x over previous
"""Pixel multi-class InfoNCE loss on 8 TRN2 NeuronCores.

Reference computation (see problem):
  e = normalize(emb.transpose(0,2,3,1).reshape(N, C))       # N=8192, C=128
  logits = (e @ e.T) / 0.1, diag masked to -inf
  row_loss = -log(pos_sum / (tot_sum + eps))                 # per row
  loss = mean over present classes of per-class mean row_loss

Sharding: each of 8 cores owns 1024 rows (i) and computes its
[8192(j), 1024(i)] block of exp(logits^T) against the full embedding,
reducing per-class sums on the fly via a one-hot matmul.  Per-core
output is the per-class partial sum of row losses [4]; the host sums
partials and divides by class counts.

Numerics: embeddings are used in bf16; normalization scales are applied
to the 1024 local columns (rhs) on-device, while the 8192-column (lhsT)
side stays raw bf16 and its per-j scale is folded into the Exp
activation as a per-partition scale vector.  The diagonal exp(self
logit) is recomputed through the identical bf16/f32 path and subtracted
from the pos/tot sums, which is exact up to f32 rounding.
"""

import math

import ml_dtypes
import numpy as np
from contextlib import ExitStack

import concourse.bass as bass
import concourse.tile as tile
from concourse import bacc, mybir
from concourse.bass_utils import run_bass_kernel_spmd

F32 = mybir.dt.float32
BF16 = mybir.dt.bfloat16

N = 8192
C = 128
NCORES = 8
RPC = N // NCORES  # 1024 rows (i) per core
NCLS = 4
EPS = 1e-6
# rsqrt with the 1/temperature fold: s = sqrt(10/x) = exp(-0.5*ln(x) + 0.5*ln(10))
RSQRT_BIAS = 0.5 * math.log(10.0)

_NC_CACHE = []


def _patch_act_tables():
    """Keep Exp and Ln only in the combined natural_log_exp_and_others set,
    so the ACT table is loaded once instead of ping-ponging between the
    exp-only and ln-only sets (1.3us + drain per switch)."""
    if getattr(bacc, "_act_tables_patched", False):
        return
    orig = bacc.get_activation_tables

    def patched(arch):
        t = orig(arch)
        exp_ln = {mybir.ActivationFunctionType.Exp, mybir.ActivationFunctionType.Ln}
        out = {}
        for name, funcs in t.items():
            if name != "natural_log_exp_and_others":
                funcs = set(funcs) - exp_ln
            out[name] = funcs
        return out

    bacc.get_activation_tables = patched
    bacc._act_tables_patched = True


def _build_body(tc, et, rows, ohj, ohi, cls):
    nc = tc.nc
    ctx = ExitStack()
    with ctx:
        pc = ctx.enter_context(tc.tile_pool(name="const", bufs=1))
        pbig = ctx.enter_context(tc.tile_pool(name="big", bufs=1))
        pex = ctx.enter_context(tc.tile_pool(name="ex", bufs=3))
        psm = ctx.enter_context(tc.tile_pool(name="sm", bufs=2))

        ones_bf = pc.tile([128, 1], BF16)
        nc.vector.memset(ones_bf[:], 1.0)
        ones_f = pc.tile([128, 1], F32)
        nc.vector.memset(ones_f[:], 1.0)
        ones_row = pc.tile([1, 128], F32)
        nc.vector.memset(ones_row[:], 1.0)
        ones_row_bf = pc.tile([1, 128], BF16)
        nc.vector.memset(ones_row_bf[:], 1.0)
        bias_t = pc.tile([128, 1], F32)
        nc.vector.memset(bias_t[:], RSQRT_BIAS)

        ohj_sb = pc.tile([128, NCLS * (N // 128)], BF16)
        nc.sync.dma_start(ohj_sb[:], ohj[:])
        ohi_sb = pc.tile([NCLS, RPC], F32)
        nc.sync.dma_start(ohi_sb[:], ohi[:])

        eb = pc.tile([C, N], BF16)        # raw E^T in bf16 (all columns)
        sT = pc.tile([128, N // 128], F32)  # per-j scale, transposed layout
        erows = pc.tile([C, RPC], BF16)   # normalized rows slice (rhs)
        dexp = pc.tile([1, RPC], F32)     # exp of diag logit per local row
        cls_parts = pc.tile([NCLS, 2], F32)

        # ---------------- setup: all-columns side ----------------
        with tc.tile_pool(name="pssetup", bufs=2, space="PSUM") as pss:
            sqb = pbig.tile([C, N], BF16)
            ps_ssq = pss.tile([128, N // 128], F32, tag="psq")
            CK = 1024
            for k in range(N // CK):
                sl = slice(k * CK, (k + 1) * CK)
                nc.sync.dma_start(eb[:, sl], et[:, sl])
                nc.vector.tensor_mul(sqb[:, sl], eb[:, sl], eb[:, sl])
                for jb in range(k * CK // 128, (k + 1) * CK // 128):
                    nc.tensor.matmul(ps_ssq[:, jb:jb + 1],
                                     sqb[:, jb * 128:(jb + 1) * 128], ones_bf[:],
                                     start=True, stop=True)
            ssqT = pbig.tile([128, N // 128], F32)
            nc.vector.tensor_copy(ssqT[:], ps_ssq[:])
            nc.scalar.activation(sT[:], ssqT[:], mybir.ActivationFunctionType.Ln)
            nc.scalar.activation(sT[:], sT[:], mybir.ActivationFunctionType.Exp,
                                 scale=-0.5, bias=bias_t[:])

            # ---------------- setup: rows side ----------------
            ebr = pbig.tile([C, RPC], BF16)
            nc.sync.dma_start(ebr[:], rows[:])
            sqr = pbig.tile([C, RPC], BF16)
            nc.vector.tensor_mul(sqr[:], ebr[:], ebr[:])

            ssq_row = pbig.tile([1, RPC], F32)
            for t in range(RPC // 512):
                ps = pss.tile([1, 512], F32, tag="psrow")
                nc.tensor.matmul(ps[:], ones_bf[:], sqr[:, t * 512:(t + 1) * 512],
                                 start=True, stop=True)
                nc.vector.tensor_copy(ssq_row[:, t * 512:(t + 1) * 512], ps[:])
            s_row = pbig.tile([1, RPC], F32)
            nc.scalar.activation(s_row[:], ssq_row[:], mybir.ActivationFunctionType.Ln)
            nc.scalar.activation(s_row[:], s_row[:], mybir.ActivationFunctionType.Exp,
                                 scale=-0.5, bias=bias_t[0:1])

            # broadcast s_row across partitions via K=1 matmul, normalize rows
            for t in range(RPC // 512):
                sl = slice(t * 512, (t + 1) * 512)
                psb = pss.tile([128, 512], F32, tag="psb")
                nc.tensor.matmul(psb[:], ones_row[:], s_row[:, sl], start=True, stop=True)
                nc.vector.tensor_mul(erows[:, sl], ebr[:, sl], psb[:])

            # diag logit: (sum_c b(raw)*b(e')) * s_i, through the same rounding
            mix = pbig.tile([C, RPC], F32)
            nc.vector.tensor_mul(mix[:], ebr[:], erows[:])
            dv = pbig.tile([1, RPC], F32)
            for t in range(RPC // 512):
                sl = slice(t * 512, (t + 1) * 512)
                pd = pss.tile([1, 512], F32, tag="pd")
                nc.tensor.matmul(pd[:], ones_f[:], mix[:, sl], start=True, stop=True)
                nc.vector.tensor_copy(dv[:, sl], pd[:])
            dvs = pbig.tile([1, RPC], F32)
            nc.vector.tensor_mul(dvs[:], dv[:], s_row[:])
            dexp_b = pbig.tile([1, RPC], BF16)
            nc.scalar.activation(dexp_b[:], dvs[:], mybir.ActivationFunctionType.Exp)
            nc.vector.tensor_copy(dexp[:], dexp_b[:])

        # ---------------- main loop ----------------
        ssb = psm.tile([NCLS, RPC], F32, tag="ssb")
        with tc.tile_pool(name="pslg", bufs=2, space="PSUM") as pslg, \
             tc.tile_pool(name="psacc", bufs=1, space="PSUM") as psacc:
            s_acc = psacc.tile([NCLS, RPC], F32)
            nj = N // 128
            for jb in range(nj):
                lg = pslg.tile([128, RPC], F32, tag="lg")
                exb = pex.tile([128, RPC], BF16, tag="exb")
                for it in range(RPC // 512):
                    sl = slice(it * 512, (it + 1) * 512)
                    nc.tensor.matmul(lg[:, sl], eb[:, jb * 128:(jb + 1) * 128],
                                     erows[:, sl], start=True, stop=True)
                nc.scalar.activation(exb[:], lg[:], mybir.ActivationFunctionType.Exp,
                                     scale=sT[:, jb:jb + 1])
                for it in range(RPC // 512):
                    sl = slice(it * 512, (it + 1) * 512)
                    nc.tensor.matmul(s_acc[:, sl], ohj_sb[:, jb * NCLS:(jb + 1) * NCLS],
                                     exb[:, sl], start=(jb == 0), stop=(jb == nj - 1),
                                     skip_group_check=True)
            nc.vector.tensor_copy(ssb[:], s_acc[:])

        # ---------------- tail ----------------
        with tc.tile_pool(name="pstail", bufs=2, space="PSUM") as pstail:
            for it in range(RPC // 512):
                sl = slice(it * 512, (it + 1) * 512)
                pm = psm.tile([NCLS, 512], F32, tag="pm")
                nc.vector.tensor_mul(pm[:], ssb[:, sl], ohi_sb[:, sl])
                # sum the 4 class rows via K=4 ones-matmul (DVE cannot read
                # partition offsets 1..3)
                tot_ps = pstail.tile([1, 512], F32, tag="totps")
                nc.tensor.matmul(tot_ps[:], ones_f[0:NCLS, 0:1], ssb[:, sl],
                                 start=True, stop=True)
                pos_ps = pstail.tile([1, 512], F32, tag="posps")
                nc.tensor.matmul(pos_ps[:], ones_f[0:NCLS, 0:1], pm[:],
                                 start=True, stop=True)
                # remove diag term, add eps to denominator
                pos = psm.tile([1, 512], F32, tag="pos")
                tot = psm.tile([1, 512], F32, tag="tot")
                nc.vector.tensor_sub(pos[:], pos_ps[:], dexp[:, sl])
                nc.vector.tensor_sub(tot[:], tot_ps[:], dexp[:, sl])
                nc.vector.tensor_scalar_add(tot[:], tot[:], EPS)
                lt = psm.tile([1, 512], F32, tag="lt")
                lp = psm.tile([1, 512], F32, tag="lp")
                nc.scalar.activation(lt[:], tot[:], mybir.ActivationFunctionType.Ln)
                nc.scalar.activation(lp[:], pos[:], mybir.ActivationFunctionType.Ln)
                rl = psm.tile([1, 512], BF16, tag="rl")
                nc.vector.tensor_sub(rl[:], lt[:], lp[:])
                # broadcast row_loss to 4 partitions, mask per class, reduce over i
                p4 = pstail.tile([NCLS, 512], F32, tag="p4")
                nc.tensor.matmul(p4[:], ones_row_bf[0:1, 0:NCLS], rl[:],
                                 start=True, stop=True)
                cm = psm.tile([NCLS, 512], F32, tag="cm")
                nc.vector.tensor_mul(cm[:], p4[:], ohi_sb[:, sl])
                nc.vector.reduce_sum(cls_parts[:, it:it + 1], cm[:],
                                     axis=mybir.AxisListType.X)
            cls_f = psm.tile([NCLS, 1], F32, tag="clsf")
            nc.vector.tensor_add(cls_f[:], cls_parts[:, 0:1], cls_parts[:, 1:2])
            nc.sync.dma_start(cls[:], cls_f[:])


def build():
    import os
    if not os.environ.get('NO_ACT_PATCH'):
        _patch_act_tables()
    nc = bacc.Bacc("TRN2", target_bir_lowering=False, debug=False)
    et = nc.dram_tensor("et", [C, N], BF16, kind="ExternalInput").ap()
    rows = nc.dram_tensor("rows", [C, RPC], BF16, kind="ExternalInput").ap()
    ohj = nc.dram_tensor("ohj", [128, NCLS * (N // 128)], BF16, kind="ExternalInput").ap()
    ohi = nc.dram_tensor("ohi", [NCLS, RPC], F32, kind="ExternalInput").ap()
    cls = nc.dram_tensor("cls", [NCLS, 1], F32, kind="ExternalOutput").ap()
    with tile.TileContext(nc) as tc:
        _build_body(tc, et, rows, ohj, ohi, cls)
    nc.compile()
    return nc


def _get_nc():
    if not _NC_CACHE:
        _NC_CACHE.append(build())
    return _NC_CACHE[0]


def make_in_maps(emb, labels):
    emb = np.asarray(emb, dtype=np.float32)
    lab = np.asarray(labels).astype(np.int64).reshape(-1)
    assert emb.shape == (2, C, 64, 64) and lab.shape == (N,)
    et = emb.reshape(2, C, N // 2)
    et = np.ascontiguousarray(
        np.concatenate([et[0], et[1]], axis=1)
    ).astype(ml_dtypes.bfloat16)  # [C, N]
    oh = (lab[:, None] == np.arange(NCLS)[None, :]).astype(np.float32)  # [N, 4]
    ohj = np.ascontiguousarray(
        oh.reshape(N // 128, 128, NCLS).transpose(1, 0, 2).reshape(128, -1)
    ).astype(ml_dtypes.bfloat16)
    in_maps = []
    for r in range(NCORES):
        sl = slice(r * RPC, (r + 1) * RPC)
        in_maps.append({
            "et": et,
            "rows": np.ascontiguousarray(et[:, sl]),
            "ohj": ohj,
            "ohi": np.ascontiguousarray(oh[sl].T),
        })
    return in_maps, lab


def finish(results, lab):
    sums = np.zeros(NCLS, np.float64)
    for r in results:
        sums += r["cls"].reshape(NCLS).astype(np.float64)
    counts = np.bincount(lab, minlength=NCLS).astype(np.float64)
    present = counts > 0
    per_cls = np.where(present, sums / np.maximum(counts, 1.0), 0.0)
    return np.float32(per_cls.sum() / present.sum())


def kernel(emb, labels):
    in_maps, lab = make_in_maps(emb, labels)
    nc = _get_nc()
    res = run_bass_kernel_spmd(nc, in_maps, core_ids=list(range(NCORES)))
    return finish(res.results, lab)


# revision 10
# speedup vs baseline: 1.1443x; 1.1045x over previous
"""Pixel multi-class InfoNCE loss on 8 TRN2 NeuronCores.

Reference computation (see problem):
  e = normalize(emb.transpose(0,2,3,1).reshape(N, C))       # N=8192, C=128
  logits = (e @ e.T) / 0.1, diag masked to -inf
  row_loss = -log(pos_sum / (tot_sum + eps))                 # per row
  loss = mean over present classes of per-class mean row_loss

Sharding: each of 8 cores owns 1024 rows (i) and computes its
[8192(j), 1024(i)] block of exp(logits^T) against the full embedding,
reducing per-class sums on the fly via a one-hot matmul.  Per-core
output is the per-class partial sum of row losses [4]; the host sums
partials and divides by class counts.

Numerics: embeddings are used in bf16; normalization scales are applied
to the 1024 local columns (rhs) on-device, while the 8192-column (lhsT)
side stays raw bf16 and its per-j scale is folded into the Exp
activation as a per-partition scale vector.  The diagonal exp(self
logit) is recomputed through the identical bf16/f32 path and subtracted
from the pos/tot sums, which is exact up to f32 rounding.
"""

import math

import ml_dtypes
import numpy as np
from contextlib import ExitStack

import concourse.bass as bass
import concourse.tile as tile
from concourse import bacc, mybir
from concourse.bass_utils import run_bass_kernel_spmd

F32 = mybir.dt.float32
BF16 = mybir.dt.bfloat16

N = 8192
C = 128
NCORES = 8
RPC = N // NCORES  # 1024 rows (i) per core
NCLS = 4
EPS = 1e-6
# rsqrt with the 1/temperature fold: s = sqrt(10/x) = exp(-0.5*ln(x) + 0.5*ln(10))
RSQRT_BIAS = 0.5 * math.log(10.0)

_NC_CACHE = []


def _patch_act_tables():
    """Keep Exp and Ln only in the combined natural_log_exp_and_others set,
    so the ACT table is loaded once instead of ping-ponging between the
    exp-only and ln-only sets (1.3us + drain per switch)."""
    if getattr(bacc, "_act_tables_patched", False):
        return
    orig = bacc.get_activation_tables

    def patched(arch):
        t = orig(arch)
        exp_ln = {mybir.ActivationFunctionType.Exp, mybir.ActivationFunctionType.Ln}
        out = {}
        for name, funcs in t.items():
            if name != "natural_log_exp_and_others":
                funcs = set(funcs) - exp_ln
            out[name] = funcs
        return out

    bacc.get_activation_tables = patched
    bacc._act_tables_patched = True


def _build_body(tc, et, rows, ohj, ohi, cls):
    nc = tc.nc
    ctx = ExitStack()
    with ctx:
        pc = ctx.enter_context(tc.tile_pool(name="const", bufs=1))
        pbig = ctx.enter_context(tc.tile_pool(name="big", bufs=1))
        pex = ctx.enter_context(tc.tile_pool(name="ex", bufs=3))
        psm = ctx.enter_context(tc.tile_pool(name="sm", bufs=2))

        ones_bf = pc.tile([128, 1], BF16)
        nc.vector.memset(ones_bf[:], 1.0)
        ones_f = pc.tile([128, 1], F32)
        nc.vector.memset(ones_f[:], 1.0)
        ones_row = pc.tile([1, 128], F32)
        nc.vector.memset(ones_row[:], 1.0)
        ones_row_bf = pc.tile([1, 128], BF16)
        nc.vector.memset(ones_row_bf[:], 1.0)
        bias_t = pc.tile([128, 1], F32)
        nc.vector.memset(bias_t[:], RSQRT_BIAS)

        ohj_sb = pc.tile([128, NCLS * (N // 128)], BF16)
        nc.sync.dma_start(ohj_sb[:], ohj[:])
        ohi_sb = pc.tile([NCLS, RPC], F32)
        nc.sync.dma_start(ohi_sb[:], ohi[:])

        eb = pc.tile([C, N], BF16)        # raw E^T in bf16 (all columns)
        sT = pc.tile([128, N // 128], F32)  # per-j scale, transposed layout
        erows = pc.tile([C, RPC], BF16)   # normalized rows slice (rhs)
        dexp = pc.tile([1, RPC], F32)     # exp of diag logit per local row
        cls_parts = pc.tile([NCLS, 2], F32)

        # ---------------- setup: rows side (first — it gates the main MMs) ----------------
        with tc.tile_pool(name="pssetup", bufs=2, space="PSUM") as pss:
            ebr = pbig.tile([C, RPC], BF16)
            nc.sync.dma_start(ebr[:], rows[:])
            sqr = pbig.tile([C, RPC], BF16)
            nc.vector.tensor_mul(sqr[:], ebr[:], ebr[:])

            ssq_row = pbig.tile([1, RPC], F32)
            for t in range(RPC // 512):
                ps = pss.tile([1, 512], F32, tag="psrow")
                nc.tensor.matmul(ps[:], ones_bf[:], sqr[:, t * 512:(t + 1) * 512],
                                 start=True, stop=True)
                nc.vector.tensor_copy(ssq_row[:, t * 512:(t + 1) * 512], ps[:])
            s_row = pbig.tile([1, RPC], F32)
            nc.scalar.activation(s_row[:], ssq_row[:], mybir.ActivationFunctionType.Ln)
            nc.scalar.activation(s_row[:], s_row[:], mybir.ActivationFunctionType.Exp,
                                 scale=-0.5, bias=bias_t[0:1])

            # broadcast s_row across partitions via K=1 matmul, normalize rows
            for t in range(RPC // 512):
                sl = slice(t * 512, (t + 1) * 512)
                psb = pss.tile([128, 512], F32, tag="psb")
                nc.tensor.matmul(psb[:], ones_row[:], s_row[:, sl], start=True, stop=True)
                nc.vector.tensor_mul(erows[:, sl], ebr[:, sl], psb[:])

            # ---------------- setup: all-columns side ----------------
            sqb = pbig.tile([C, N], BF16)
            ps_ssq = pss.tile([128, N // 128], F32, tag="psq")
            CK = 1024
            for k in range(N // CK):
                sl = slice(k * CK, (k + 1) * CK)
                nc.sync.dma_start(eb[:, sl], et[:, sl])
                nc.vector.tensor_mul(sqb[:, sl], eb[:, sl], eb[:, sl])
                for jb in range(k * CK // 128, (k + 1) * CK // 128):
                    nc.tensor.matmul(ps_ssq[:, jb:jb + 1],
                                     sqb[:, jb * 128:(jb + 1) * 128], ones_bf[:],
                                     start=True, stop=True)
            ssqT = pbig.tile([128, N // 128], F32)
            nc.vector.tensor_copy(ssqT[:], ps_ssq[:])
            nc.scalar.activation(sT[:], ssqT[:], mybir.ActivationFunctionType.Ln)
            nc.scalar.activation(sT[:], sT[:], mybir.ActivationFunctionType.Exp,
                                 scale=-0.5, bias=bias_t[:])

            # diag logit: (sum_c b(raw)*b(e')) * s_i, through the same rounding
            mix = pbig.tile([C, RPC], F32)
            nc.vector.tensor_mul(mix[:], ebr[:], erows[:])
            dv = pbig.tile([1, RPC], F32)
            for t in range(RPC // 512):
                sl = slice(t * 512, (t + 1) * 512)
                pd = pss.tile([1, 512], F32, tag="pd")
                nc.tensor.matmul(pd[:], ones_f[:], mix[:, sl], start=True, stop=True)
                nc.vector.tensor_copy(dv[:, sl], pd[:])
            dvs = pbig.tile([1, RPC], F32)
            nc.vector.tensor_mul(dvs[:], dv[:], s_row[:])
            dexp_b = pbig.tile([1, RPC], BF16)
            nc.scalar.activation(dexp_b[:], dvs[:], mybir.ActivationFunctionType.Exp)
            nc.vector.tensor_copy(dexp[:], dexp_b[:])

        # ---------------- main loop ----------------
        # Software-pipelined: the class matmul of block jb is emitted after
        # the main matmuls of block jb+1, so the PE (strict program order)
        # never stalls waiting for the Exp of jb.
        ssb = psm.tile([NCLS, RPC], F32, tag="ssb")
        with tc.tile_pool(name="pslg", bufs=3, space="PSUM") as pslg, \
             tc.tile_pool(name="psacc", bufs=1, space="PSUM") as psacc:
            s_acc = psacc.tile([NCLS, RPC], F32)
            nj = N // 128
            exbs = {}

            def emit_main(jb):
                lg = pslg.tile([128, RPC], F32, tag="lg")
                exb = pex.tile([128, RPC], BF16, tag="exb")
                exbs[jb] = exb
                for it in range(RPC // 512):
                    sl = slice(it * 512, (it + 1) * 512)
                    nc.tensor.matmul(lg[:, sl], eb[:, jb * 128:(jb + 1) * 128],
                                     erows[:, sl], start=True, stop=True)
                nc.scalar.activation(exb[:], lg[:], mybir.ActivationFunctionType.Exp,
                                     scale=sT[:, jb:jb + 1])

            def emit_cls(jb):
                exb = exbs.pop(jb)
                for it in range(RPC // 512):
                    sl = slice(it * 512, (it + 1) * 512)
                    nc.tensor.matmul(s_acc[:, sl], ohj_sb[:, jb * NCLS:(jb + 1) * NCLS],
                                     exb[:, sl], start=(jb == 0), stop=(jb == nj - 1),
                                     skip_group_check=True)

            emit_main(0)
            for jb in range(1, nj):
                emit_main(jb)
                emit_cls(jb - 1)
            emit_cls(nj - 1)
            nc.vector.tensor_copy(ssb[:], s_acc[:])

        # ---------------- tail ----------------
        with tc.tile_pool(name="pstail", bufs=2, space="PSUM") as pstail:
            for it in range(RPC // 512):
                sl = slice(it * 512, (it + 1) * 512)
                pm = psm.tile([NCLS, 512], F32, tag="pm")
                nc.vector.tensor_mul(pm[:], ssb[:, sl], ohi_sb[:, sl])
                # sum the 4 class rows via K=4 ones-matmul (DVE cannot read
                # partition offsets 1..3)
                tot_ps = pstail.tile([1, 512], F32, tag="totps")
                nc.tensor.matmul(tot_ps[:], ones_f[0:NCLS, 0:1], ssb[:, sl],
                                 start=True, stop=True)
                pos_ps = pstail.tile([1, 512], F32, tag="posps")
                nc.tensor.matmul(pos_ps[:], ones_f[0:NCLS, 0:1], pm[:],
                                 start=True, stop=True)
                # remove diag term, add eps to denominator
                pos = psm.tile([1, 512], F32, tag="pos")
                tot = psm.tile([1, 512], F32, tag="tot")
                nc.vector.tensor_sub(pos[:], pos_ps[:], dexp[:, sl])
                nc.vector.tensor_sub(tot[:], tot_ps[:], dexp[:, sl])
                nc.vector.tensor_scalar_add(tot[:], tot[:], EPS)
                lt = psm.tile([1, 512], F32, tag="lt")
                lp = psm.tile([1, 512], F32, tag="lp")
                nc.scalar.activation(lt[:], tot[:], mybir.ActivationFunctionType.Ln)
                nc.scalar.activation(lp[:], pos[:], mybir.ActivationFunctionType.Ln)
                rl = psm.tile([1, 512], BF16, tag="rl")
                nc.vector.tensor_sub(rl[:], lt[:], lp[:])
                # broadcast row_loss to 4 partitions, mask per class, reduce over i
                p4 = pstail.tile([NCLS, 512], F32, tag="p4")
                nc.tensor.matmul(p4[:], ones_row_bf[0:1, 0:NCLS], rl[:],
                                 start=True, stop=True)
                cm = psm.tile([NCLS, 512], F32, tag="cm")
                nc.vector.tensor_mul(cm[:], p4[:], ohi_sb[:, sl])
                nc.vector.reduce_sum(cls_parts[:, it:it + 1], cm[:],
                                     axis=mybir.AxisListType.X)
            cls_f = psm.tile([NCLS, 1], F32, tag="clsf")
            nc.vector.tensor_add(cls_f[:], cls_parts[:, 0:1], cls_parts[:, 1:2])
            nc.sync.dma_start(cls[:], cls_f[:])


def build():
    import os
    if not os.environ.get('NO_ACT_PATCH'):
        _patch_act_tables()
    nc = bacc.Bacc("TRN2", target_bir_lowering=False, debug=False)
    et = nc.dram_tensor("et", [C, N], BF16, kind="ExternalInput").ap()
    rows = nc.dram_tensor("rows", [C, RPC], BF16, kind="ExternalInput").ap()
    ohj = nc.dram_tensor("ohj", [128, NCLS * (N // 128)], BF16, kind="ExternalInput").ap()
    ohi = nc.dram_tensor("ohi", [NCLS, RPC], F32, kind="ExternalInput").ap()
    cls = nc.dram_tensor("cls", [NCLS, 1], F32, kind="ExternalOutput").ap()
    with tile.TileContext(nc) as tc:
        _build_body(tc, et, rows, ohj, ohi, cls)
    nc.compile()
    return nc


def _get_nc():
    if not _NC_CACHE:
        _NC_CACHE.append(build())
    return _NC_CACHE[0]


def make_in_maps(emb, labels):
    emb = np.asarray(emb, dtype=np.float32)
    lab = np.asarray(labels).astype(np.int64).reshape(-1)
    assert emb.shape == (2, C, 64, 64) and lab.shape == (N,)
    et = emb.reshape(2, C, N // 2)
    et = np.ascontiguousarray(
        np.concatenate([et[0], et[1]], axis=1)
    ).astype(ml_dtypes.bfloat16)  # [C, N]
    oh = (lab[:, None] == np.arange(NCLS)[None, :]).astype(np.float32)  # [N, 4]
    ohj = np.ascontiguousarray(
        oh.reshape(N // 128, 128, NCLS).transpose(1, 0, 2).reshape(128, -1)
    ).astype(ml_dtypes.bfloat16)
    in_maps = []
    for r in range(NCORES):
        sl = slice(r * RPC, (r + 1) * RPC)
        in_maps.append({
            "et": et,
            "rows": np.ascontiguousarray(et[:, sl]),
            "ohj": ohj,
            "ohi": np.ascontiguousarray(oh[sl].T),
        })
    return in_maps, lab


def finish(results, lab):
    sums = np.zeros(NCLS, np.float64)
    for r in results:
        sums += r["cls"].reshape(NCLS).astype(np.float64)
    counts = np.bincount(lab, minlength=NCLS).astype(np.float64)
    present = counts > 0
    per_cls = np.where(present, sums / np.maximum(counts, 1.0), 0.0)
    return np.float32(per_cls.sum() / present.sum())


def kernel(emb, labels):
    in_maps, lab = make_in_maps(emb, labels)
    nc = _get_nc()
    res = run_bass_kernel_spmd(nc, in_maps, core_ids=list(range(NCORES)))
    return finish(res.results, lab)


# revision 18
# speedup vs baseline: 1.1550x; 1.0093x over previous
"""Pixel multi-class InfoNCE loss on 8 TRN2 NeuronCores.

Reference computation (see problem):
  e = normalize(emb.transpose(0,2,3,1).reshape(N, C))       # N=8192, C=128
  logits = (e @ e.T) / 0.1, diag masked to -inf
  row_loss = -log(pos_sum / (tot_sum + eps))                 # per row
  loss = mean over present classes of per-class mean row_loss

Sharding: each of 8 cores owns 1024 rows (i) and computes its
[8192(j), 1024(i)] block of exp(logits^T) against the full embedding,
reducing per-class sums on the fly via a one-hot matmul.  Per-core
output is the per-class partial sum of row losses [4]; the host sums
partials and divides by class counts.

Numerics: embeddings are used in bf16; normalization scales are applied
to the 1024 local columns (rhs) on-device, while the 8192-column (lhsT)
side stays raw bf16 and its per-j scale is folded into the Exp
activation as a per-partition scale vector.  The diagonal exp(self
logit) is recomputed through the identical bf16/f32 path and subtracted
from the pos/tot sums, which is exact up to f32 rounding.
"""

import math

import ml_dtypes
import numpy as np
from contextlib import ExitStack

import concourse.bass as bass
import concourse.tile as tile
from concourse import bacc, bass_isa, mybir
from concourse.bass_utils import run_bass_kernel_spmd

F32 = mybir.dt.float32
BF16 = mybir.dt.bfloat16

N = 8192
C = 128
NCORES = 8
RPC = N // NCORES  # 1024 rows (i) per core
NCLS = 4
EPS = 1e-6
# rsqrt with the 1/temperature fold: s = sqrt(10/x) = exp(-0.5*ln(x) + 0.5*ln(10))
RSQRT_BIAS = 0.5 * math.log(10.0)

_NC_CACHE = []
_ACT_ROOT = []


def _make_act_root():
    """Copy the PWP activation-table dir and strip exp/ln from every set
    except natural_log_exp_and_others, so walrus's lower_act maps both
    functions to the one combined set (ids preserved by keeping order)."""
    if _ACT_ROOT:
        return
    import json as _json
    import os
    import shutil
    import tempfile
    from neuronxcc.driver.Job import Job
    from neuronxcc.driver.jobs.support.FindActInfo import findActInfoFile

    src_json = findActInfoFile(Job.getPackageDir(), "gen3")
    dst = tempfile.mkdtemp(prefix="act_root_")
    for f in os.listdir(os.path.dirname(src_json)):
        fp = os.path.join(os.path.dirname(src_json), f)
        if os.path.isfile(fp):
            shutil.copy(fp, dst)
    p = os.path.join(dst, os.path.basename(src_json))
    with open(p) as fh:
        info = _json.load(fh)
    for s in info["act_func_sets"]:
        if s["name"] != "natural_log_exp_and_others":
            s["act"].pop("exp", None)
            s["act"].pop("ln", None)
    with open(p, "w") as fh:
        _json.dump(info, fh)
    os.environ["BASS_ACT_ROOT_JSON_PATH"] = p
    _ACT_ROOT.append(p)


def _patch_act_tables():
    """Keep Exp and Ln only in the combined natural_log_exp_and_others set,
    so the ACT table is loaded once instead of ping-ponging between the
    exp-only and ln-only sets (1.3us + drain per switch)."""
    if getattr(bacc, "_act_tables_patched", False):
        return
    orig = bacc.get_activation_tables

    def patched(arch):
        t = orig(arch)
        exp_ln = {mybir.ActivationFunctionType.Exp, mybir.ActivationFunctionType.Ln}
        out = {}
        for name, funcs in t.items():
            if name != "natural_log_exp_and_others":
                funcs = set(funcs) - exp_ln
            out[name] = funcs
        return out

    bacc.get_activation_tables = patched
    bacc._act_tables_patched = True


def _build_body(tc, et, rows, ohj, ohi, cls):
    nc = tc.nc
    ctx = ExitStack()
    with ctx:
        pc = ctx.enter_context(tc.tile_pool(name="const", bufs=1))
        pbig = ctx.enter_context(tc.tile_pool(name="big", bufs=1))
        pex = ctx.enter_context(tc.tile_pool(name="ex", bufs=3))
        psm = ctx.enter_context(tc.tile_pool(name="sm", bufs=2))

        ones_bf = pc.tile([128, 1], BF16)
        nc.vector.memset(ones_bf[:], 1.0)
        ones_f = pc.tile([128, 1], F32)
        nc.vector.memset(ones_f[:], 1.0)
        ones_row = pc.tile([1, 128], F32)
        nc.vector.memset(ones_row[:], 1.0)
        ones_row_bf = pc.tile([1, 128], BF16)
        nc.vector.memset(ones_row_bf[:], 1.0)
        bias_t = pc.tile([128, 1], F32)
        nc.vector.memset(bias_t[:], RSQRT_BIAS)

        ohj_sb = pc.tile([128, (NCLS + 1) * (N // 128)], BF16)
        nc.sync.dma_start(ohj_sb[:], ohj[:])
        ohi_sb = pc.tile([NCLS, RPC], F32)
        nc.sync.dma_start(ohi_sb[:], ohi[:])

        eb = pc.tile([C, N], BF16)        # raw E^T in bf16 (all columns)
        sT = pc.tile([128, N // 128], F32)  # per-j scale, transposed layout
        erows = pc.tile([C, RPC], BF16)   # normalized rows slice (rhs)
        dexp = pc.tile([1, RPC], F32)     # exp of diag logit per local row

        # ---------------- setup: rows side (first — it gates the main MMs) ----------------
        with tc.tile_pool(name="pssetup", bufs=2, space="PSUM") as pss:
            ebr = pbig.tile([C, RPC], BF16)
            nc.sync.dma_start(ebr[:], rows[:])
            sqr = pbig.tile([C, RPC], BF16)
            nc.vector.tensor_mul(sqr[:], ebr[:], ebr[:])

            ssq_row = pbig.tile([1, RPC], F32)
            for t in range(RPC // 512):
                ps = pss.tile([1, 512], F32, tag="psrow")
                nc.tensor.matmul(ps[:], ones_bf[:], sqr[:, t * 512:(t + 1) * 512],
                                 start=True, stop=True)
                nc.vector.tensor_copy(ssq_row[:, t * 512:(t + 1) * 512], ps[:])
            s_row = pbig.tile([1, RPC], F32)
            nc.scalar.activation(s_row[:], ssq_row[:], mybir.ActivationFunctionType.Ln)
            nc.scalar.activation(s_row[:], s_row[:], mybir.ActivationFunctionType.Exp,
                                 scale=-0.5, bias=bias_t[0:1])

            # broadcast s_row across partitions via K=1 matmul, normalize rows
            for t in range(RPC // 512):
                sl = slice(t * 512, (t + 1) * 512)
                psb = pss.tile([128, 512], F32, tag="psb")
                nc.tensor.matmul(psb[:], ones_row[:], s_row[:, sl], start=True, stop=True)
                nc.vector.tensor_mul(erows[:, sl], ebr[:, sl], psb[:])

            # ---------------- setup: all-columns side ----------------
            sqb = pbig.tile([C, N], BF16)
            ps_ssq = pss.tile([128, N // 128], F32, tag="psq")
            CK = 1024
            for k in range(N // CK):
                sl = slice(k * CK, (k + 1) * CK)
                nc.sync.dma_start(eb[:, sl], et[:, sl])
                nc.vector.tensor_mul(sqb[:, sl], eb[:, sl], eb[:, sl])
                for jb in range(k * CK // 128, (k + 1) * CK // 128):
                    nc.tensor.matmul(ps_ssq[:, jb:jb + 1],
                                     sqb[:, jb * 128:(jb + 1) * 128], ones_bf[:],
                                     start=True, stop=True)
            ssqT = pbig.tile([128, N // 128], F32)
            nc.vector.tensor_copy(ssqT[:], ps_ssq[:])
            nc.scalar.activation(sT[:], ssqT[:], mybir.ActivationFunctionType.Ln)
            nc.scalar.activation(sT[:], sT[:], mybir.ActivationFunctionType.Exp,
                                 scale=-0.5, bias=bias_t[:])

            # diag logit: (sum_c b(raw)*b(e')) * s_i, through the same rounding
            mix = pbig.tile([C, RPC], F32)
            nc.vector.tensor_mul(mix[:], ebr[:], erows[:])
            dv = pbig.tile([1, RPC], F32)
            for t in range(RPC // 512):
                sl = slice(t * 512, (t + 1) * 512)
                pd = pss.tile([1, 512], F32, tag="pd")
                nc.tensor.matmul(pd[:], ones_f[:], mix[:, sl], start=True, stop=True)
                nc.vector.tensor_copy(dv[:, sl], pd[:])
            dvs = pbig.tile([1, RPC], F32)
            nc.vector.tensor_mul(dvs[:], dv[:], s_row[:])
            dexp_b = pbig.tile([1, RPC], BF16)
            nc.scalar.activation(dexp_b[:], dvs[:], mybir.ActivationFunctionType.Exp)
            nc.vector.tensor_copy(dexp[:], dexp_b[:])

        # ---------------- main loop ----------------
        # Software-pipelined: the class matmul of block jb is emitted after
        # the main matmuls of block jb+1, so the PE (strict program order)
        # never stalls waiting for the Exp of jb.
        ssb = psm.tile([NCLS + 1, RPC], F32, tag="ssb")
        pm = psm.tile([NCLS, RPC], F32, tag="pm")
        tot_sb = psm.tile([1, RPC], F32, tag="totsb")
        with tc.tile_pool(name="pslg", bufs=3, space="PSUM") as pslg, \
             tc.tile_pool(name="psacc", bufs=1, space="PSUM") as psacc:
            s_acc = psacc.tile([NCLS + 1, RPC], F32)
            nj = N // 128
            exbs = {}

            def emit_main(jb):
                lg = pslg.tile([128, RPC], F32, tag="lg")
                exb = pex.tile([128, RPC], BF16, tag="exb")
                exbs[jb] = exb
                for it in range(RPC // 512):
                    sl = slice(it * 512, (it + 1) * 512)
                    nc.tensor.matmul(lg[:, sl], eb[:, jb * 128:(jb + 1) * 128],
                                     erows[:, sl], start=True, stop=True)
                nc.scalar.activation(exb[:], lg[:], mybir.ActivationFunctionType.Exp,
                                     scale=sT[:, jb:jb + 1])

            def emit_cls(jb):
                exb = exbs.pop(jb)
                for it in range(RPC // 512):
                    sl = slice(it * 512, (it + 1) * 512)
                    nc.tensor.matmul(
                        s_acc[:, sl],
                        ohj_sb[:, jb * (NCLS + 1):(jb + 1) * (NCLS + 1)],
                        exb[:, sl], start=(jb == 0), stop=(jb == nj - 1),
                        skip_group_check=True)

            emit_main(0)
            for jb in range(1, nj):
                emit_main(jb)
                emit_cls(jb - 1)
            emit_cls(nj - 1)
            nc.vector.tensor_copy(ssb[:], s_acc[:])
            nc.vector.tensor_mul(pm[:], s_acc[0:NCLS, :], ohi_sb[:])
        # row 4 of ssb is the total sum; DMA moves it to partition 0
        # (engines cannot read partition offsets that are not multiples of 32)
        nc.sync.dma_start(tot_sb[:], ssb[NCLS:NCLS + 1, :])

        # ---------------- tail ----------------
        with tc.tile_pool(name="pstail", bufs=1, space="PSUM") as pstail:
            p4 = pstail.tile([NCLS, RPC], F32)
            for it in range(RPC // 512):
                sl = slice(it * 512, (it + 1) * 512)
                pos_ps = pstail.tile([1, 512], F32, tag="posps")
                nc.tensor.matmul(pos_ps[:], ones_f[0:NCLS, 0:1], pm[:, sl],
                                 start=True, stop=True)
                # remove diag term; add eps to the denominator
                pos = psm.tile([1, 512], F32, tag="pos")
                tot = psm.tile([1, 512], F32, tag="tot")
                nc.vector.tensor_sub(pos[:], pos_ps[:], dexp[:, sl])
                nc.vector.tensor_sub(tot[:], tot_sb[:, sl], dexp[:, sl])
                nc.vector.tensor_scalar_add(tot[:], tot[:], EPS)
                lt = psm.tile([1, 512], F32, tag="lt")
                lp = psm.tile([1, 512], F32, tag="lp")
                nc.scalar.activation(lt[:], tot[:], mybir.ActivationFunctionType.Ln)
                nc.scalar.activation(lp[:], pos[:], mybir.ActivationFunctionType.Ln)
                rl = psm.tile([1, 512], BF16, tag="rl")
                nc.vector.tensor_sub(rl[:], lt[:], lp[:])
                # broadcast row_loss to 4 partitions
                nc.tensor.matmul(p4[:, sl], ones_row_bf[0:1, 0:NCLS], rl[:],
                                 start=True, stop=True)
            # mask per class and reduce over all local i
            cm = psm.tile([NCLS, RPC], F32, tag="cm")
            cls_f = psm.tile([NCLS, 1], F32, tag="clsf")
            nc.vector.tensor_mul(cm[:], p4[:], ohi_sb[:])
            nc.vector.reduce_sum(cls_f[:], cm[:], axis=mybir.AxisListType.X)
            nc.sync.dma_start(cls[:], cls_f[:])


def build():
    import os
    if not os.environ.get('NO_ACT_PATCH'):
        _patch_act_tables()
        _make_act_root()
    nc = bacc.Bacc("TRN2", target_bir_lowering=False, debug=False)
    et = nc.dram_tensor("et", [C, N], BF16, kind="ExternalInput").ap()
    rows = nc.dram_tensor("rows", [C, RPC], BF16, kind="ExternalInput").ap()
    ohj = nc.dram_tensor("ohj", [128, (NCLS + 1) * (N // 128)], BF16, kind="ExternalInput").ap()
    ohi = nc.dram_tensor("ohi", [NCLS, RPC], F32, kind="ExternalInput").ap()
    cls = nc.dram_tensor("cls", [NCLS, 1], F32, kind="ExternalOutput").ap()
    with tile.TileContext(nc) as tc:
        _build_body(tc, et, rows, ohj, ohi, cls)
    nc.compile()
    return nc


def _get_nc():
    if not _NC_CACHE:
        _NC_CACHE.append(build())
    return _NC_CACHE[0]


def make_in_maps(emb, labels):
    emb = np.asarray(emb, dtype=np.float32)
    lab = np.asarray(labels).astype(np.int64).reshape(-1)
    assert emb.shape == (2, C, 64, 64) and lab.shape == (N,)
    et = emb.reshape(2, C, N // 2)
    et = np.ascontiguousarray(
        np.concatenate([et[0], et[1]], axis=1)
    ).astype(ml_dtypes.bfloat16)  # [C, N]
    oh = (lab[:, None] == np.arange(NCLS)[None, :]).astype(np.float32)  # [N, 4]
    oh5 = np.concatenate([oh, np.ones((N, 1), np.float32)], axis=1)  # [N, 5]
    ohj = np.ascontiguousarray(
        oh5.reshape(N // 128, 128, NCLS + 1).transpose(1, 0, 2).reshape(128, -1)
    ).astype(ml_dtypes.bfloat16)
    in_maps = []
    for r in range(NCORES):
        sl = slice(r * RPC, (r + 1) * RPC)
        in_maps.append({
            "et": et,
            "rows": np.ascontiguousarray(et[:, sl]),
            "ohj": ohj,
            "ohi": np.ascontiguousarray(oh[sl].T),
        })
    return in_maps, lab


def finish(results, lab):
    sums = np.zeros(NCLS, np.float64)
    for r in results:
        sums += r["cls"].reshape(NCLS).astype(np.float64)
    counts = np.bincount(lab, minlength=NCLS).astype(np.float64)
    present = counts > 0
    per_cls = np.where(present, sums / np.maximum(counts, 1.0), 0.0)
    return np.float32(per_cls.sum() / present.sum())


def kernel(emb, labels):
    in_maps, lab = make_in_maps(emb, labels)
    nc = _get_nc()
    res = run_bass_kernel_spmd(nc, in_maps, core_ids=list(range(NCORES)))
    return finish(res.results, lab)


# revision 22
# speedup vs baseline: 1.1637x; 1.0076x over previous
"""Pixel multi-class InfoNCE loss on 8 TRN2 NeuronCores.

Reference computation (see problem):
  e = normalize(emb.transpose(0,2,3,1).reshape(N, C))       # N=8192, C=128
  logits = (e @ e.T) / 0.1, diag masked to -inf
  row_loss = -log(pos_sum / (tot_sum + eps))                 # per row
  loss = mean over present classes of per-class mean row_loss

Sharding: each of 8 cores owns 1024 rows (i) and computes its
[8192(j), 1024(i)] block of exp(logits^T) against the full embedding,
reducing per-class sums on the fly via a one-hot matmul.  Per-core
output is the per-class partial sum of row losses [4]; the host sums
partials and divides by class counts.

Numerics: embeddings are used in bf16; normalization scales are applied
to the 1024 local columns (rhs) on-device, while the 8192-column (lhsT)
side stays raw bf16 and its per-j scale is folded into the Exp
activation as a per-partition scale vector.  The diagonal exp(self
logit) is recomputed through the identical bf16/f32 path and subtracted
from the pos/tot sums, which is exact up to f32 rounding.
"""

import math

import ml_dtypes
import numpy as np
from contextlib import ExitStack

import concourse.bass as bass
import concourse.tile as tile
from concourse import bacc, bass_isa, mybir
from concourse.bass_utils import run_bass_kernel_spmd

F32 = mybir.dt.float32
BF16 = mybir.dt.bfloat16

N = 8192
C = 128
NCORES = 8
RPC = N // NCORES  # 1024 rows (i) per core
NCLS = 4
EPS = 1e-6
# rsqrt with the 1/temperature fold: s = sqrt(10/x) = exp(-0.5*ln(x) + 0.5*ln(10))
RSQRT_BIAS = 0.5 * math.log(10.0)

_NC_CACHE = []
_ACT_ROOT = []


def _make_act_root():
    """Copy the PWP activation-table dir and strip exp/ln from every set
    except natural_log_exp_and_others, so walrus's lower_act maps both
    functions to the one combined set (ids preserved by keeping order)."""
    if _ACT_ROOT:
        return
    import json as _json
    import os
    import shutil
    import tempfile
    from neuronxcc.driver.Job import Job
    from neuronxcc.driver.jobs.support.FindActInfo import findActInfoFile

    src_json = findActInfoFile(Job.getPackageDir(), "gen3")
    dst = tempfile.mkdtemp(prefix="act_root_")
    for f in os.listdir(os.path.dirname(src_json)):
        fp = os.path.join(os.path.dirname(src_json), f)
        if os.path.isfile(fp):
            shutil.copy(fp, dst)
    p = os.path.join(dst, os.path.basename(src_json))
    with open(p) as fh:
        info = _json.load(fh)
    for s in info["act_func_sets"]:
        if s["name"] != "natural_log_exp_and_others":
            s["act"].pop("exp", None)
            s["act"].pop("ln", None)
    with open(p, "w") as fh:
        _json.dump(info, fh)
    os.environ["BASS_ACT_ROOT_JSON_PATH"] = p
    _ACT_ROOT.append(p)


def _patch_act_tables():
    """Keep Exp and Ln only in the combined natural_log_exp_and_others set,
    so the ACT table is loaded once instead of ping-ponging between the
    exp-only and ln-only sets (1.3us + drain per switch)."""
    if getattr(bacc, "_act_tables_patched", False):
        return
    orig = bacc.get_activation_tables

    def patched(arch):
        t = orig(arch)
        exp_ln = {mybir.ActivationFunctionType.Exp, mybir.ActivationFunctionType.Ln}
        out = {}
        for name, funcs in t.items():
            if name != "natural_log_exp_and_others":
                funcs = set(funcs) - exp_ln
            out[name] = funcs
        return out

    bacc.get_activation_tables = patched
    bacc._act_tables_patched = True


def _build_body(tc, et, rows, ohj, ohi, cls):
    nc = tc.nc
    ctx = ExitStack()
    with ctx:
        pc = ctx.enter_context(tc.tile_pool(name="const", bufs=1))
        pbig = ctx.enter_context(tc.tile_pool(name="big", bufs=1))
        pex = ctx.enter_context(tc.tile_pool(name="ex", bufs=3))
        psm = ctx.enter_context(tc.tile_pool(name="sm", bufs=2))

        ones_bf = pc.tile([128, 1], BF16)
        nc.vector.memset(ones_bf[:], 1.0)
        ones_f = pc.tile([128, 1], F32)
        nc.vector.memset(ones_f[:], 1.0)
        ones_row = pc.tile([1, 128], F32)
        nc.vector.memset(ones_row[:], 1.0)
        ones_row_bf = pc.tile([1, 128], BF16)
        nc.vector.memset(ones_row_bf[:], 1.0)
        bias_t = pc.tile([128, 1], F32)
        nc.vector.memset(bias_t[:], RSQRT_BIAS)

        ohj_sb = pc.tile([128, (NCLS + 1) * (N // 128)], BF16)
        nc.sync.dma_start(ohj_sb[:], ohj[:])
        ohi_sb = pc.tile([NCLS, RPC], F32)
        nc.sync.dma_start(ohi_sb[:], ohi[:])

        eb = pc.tile([C, N], BF16)        # raw E^T in bf16 (all columns)
        sT = pc.tile([128, N // 128], F32)  # per-j scale, transposed layout
        erows = pc.tile([C, RPC], BF16)   # normalized rows slice (rhs)
        dexp = pc.tile([1, RPC], F32)     # exp of diag logit per local row

        # ---------------- setup: rows side (first — it gates the main MMs) ----------------
        with tc.tile_pool(name="pssetup", bufs=2, space="PSUM") as pss:
            ebr = pbig.tile([C, RPC], BF16)
            nc.sync.dma_start(ebr[:], rows[:])
            sqr = pbig.tile([C, RPC], BF16)
            nc.vector.tensor_mul(sqr[:], ebr[:], ebr[:])

            ssq_row = pbig.tile([1, RPC], F32)
            for t in range(RPC // 512):
                ps = pss.tile([1, 512], F32, tag="psrow")
                nc.tensor.matmul(ps[:], ones_bf[:], sqr[:, t * 512:(t + 1) * 512],
                                 start=True, stop=True)
                nc.vector.tensor_copy(ssq_row[:, t * 512:(t + 1) * 512], ps[:])
            s_row = pbig.tile([1, RPC], F32)
            nc.scalar.activation(s_row[:], ssq_row[:], mybir.ActivationFunctionType.Ln)
            nc.scalar.activation(s_row[:], s_row[:], mybir.ActivationFunctionType.Exp,
                                 scale=-0.5, bias=bias_t[0:1])

            # broadcast s_row across partitions via K=1 bf16 matmul (f32
            # matmuls run LOW+HIGH double passes, ~2.1us each).  Round s to
            # bf16 first so the diag path sees the identical scale.
            s_rowb = pbig.tile([1, RPC], BF16)
            nc.vector.tensor_copy(s_rowb[:], s_row[:])
            for t in range(RPC // 512):
                sl = slice(t * 512, (t + 1) * 512)
                psb = pss.tile([128, 512], F32, tag="psb")
                nc.tensor.matmul(psb[:], ones_row_bf[:], s_rowb[:, sl],
                                 start=True, stop=True)
                nc.vector.tensor_mul(erows[:, sl], ebr[:, sl], psb[:])

            # ---------------- setup: all-columns side ----------------
            sqb = pbig.tile([C, N], BF16)
            ps_ssq = pss.tile([128, N // 128], F32, tag="psq")
            CK = 1024
            for k in range(N // CK):
                sl = slice(k * CK, (k + 1) * CK)
                nc.sync.dma_start(eb[:, sl], et[:, sl])
                nc.vector.tensor_mul(sqb[:, sl], eb[:, sl], eb[:, sl])
                for jb in range(k * CK // 128, (k + 1) * CK // 128):
                    nc.tensor.matmul(ps_ssq[:, jb:jb + 1],
                                     sqb[:, jb * 128:(jb + 1) * 128], ones_bf[:],
                                     start=True, stop=True)
            ssqT = pbig.tile([128, N // 128], F32)
            nc.vector.tensor_copy(ssqT[:], ps_ssq[:])
            nc.scalar.activation(sT[:], ssqT[:], mybir.ActivationFunctionType.Ln)
            nc.scalar.activation(sT[:], sT[:], mybir.ActivationFunctionType.Exp,
                                 scale=-0.5, bias=bias_t[:])

            # diag logit: (sum_c b(raw)*b(e')) * s_i.  The products are
            # split hi(bf16) + lo(residual, bf16) so two cheap bf16 matmuls
            # accumulate the exact f32 column sums (f32 matmuls cost ~2.1us
            # in LOW+HIGH passes).
            mix = pbig.tile([C, RPC], F32)
            nc.vector.tensor_mul(mix[:], ebr[:], erows[:])
            mix_hi = pbig.tile([C, RPC], BF16)
            nc.vector.tensor_copy(mix_hi[:], mix[:])
            mix_lo = pbig.tile([C, RPC], F32)
            nc.vector.tensor_sub(mix_lo[:], mix[:], mix_hi[:])
            mix_lob = pbig.tile([C, RPC], BF16)
            nc.vector.tensor_copy(mix_lob[:], mix_lo[:])
            dv = pbig.tile([1, RPC], F32)
            for t in range(RPC // 512):
                sl = slice(t * 512, (t + 1) * 512)
                pd = pss.tile([1, 512], F32, tag="pd")
                nc.tensor.matmul(pd[:], ones_bf[:], mix_hi[:, sl], start=True, stop=False)
                nc.tensor.matmul(pd[:], ones_bf[:], mix_lob[:, sl], start=False, stop=True)
                nc.vector.tensor_copy(dv[:, sl], pd[:])
            dvs = pbig.tile([1, RPC], F32)
            nc.vector.tensor_mul(dvs[:], dv[:], s_row[:])
            dexp_b = pbig.tile([1, RPC], BF16)
            nc.scalar.activation(dexp_b[:], dvs[:], mybir.ActivationFunctionType.Exp)
            nc.vector.tensor_copy(dexp[:], dexp_b[:])

        # ---------------- main loop ----------------
        # Software-pipelined: the class matmul of block jb is emitted after
        # the main matmuls of block jb+1, so the PE (strict program order)
        # never stalls waiting for the Exp of jb.
        ssb = psm.tile([NCLS + 1, RPC], F32, tag="ssb")
        pm = psm.tile([NCLS, RPC], F32, tag="pm")
        tot_sb = psm.tile([1, RPC], F32, tag="totsb")
        with tc.tile_pool(name="pslg", bufs=3, space="PSUM") as pslg, \
             tc.tile_pool(name="psacc", bufs=1, space="PSUM") as psacc:
            s_acc = psacc.tile([NCLS + 1, RPC], F32)
            nj = N // 128
            exbs = {}

            def emit_main(jb):
                lg = pslg.tile([128, RPC], F32, tag="lg")
                exb = pex.tile([128, RPC], BF16, tag="exb")
                exbs[jb] = exb
                for it in range(RPC // 512):
                    sl = slice(it * 512, (it + 1) * 512)
                    nc.tensor.matmul(lg[:, sl], eb[:, jb * 128:(jb + 1) * 128],
                                     erows[:, sl], start=True, stop=True)
                nc.scalar.activation(exb[:], lg[:], mybir.ActivationFunctionType.Exp,
                                     scale=sT[:, jb:jb + 1])

            def emit_cls(jb):
                exb = exbs.pop(jb)
                for it in range(RPC // 512):
                    sl = slice(it * 512, (it + 1) * 512)
                    nc.tensor.matmul(
                        s_acc[:, sl],
                        ohj_sb[:, jb * (NCLS + 1):(jb + 1) * (NCLS + 1)],
                        exb[:, sl], start=(jb == 0), stop=(jb == nj - 1),
                        skip_group_check=True)

            emit_main(0)
            for jb in range(1, nj):
                emit_main(jb)
                emit_cls(jb - 1)
            emit_cls(nj - 1)
            nc.scalar.copy(ssb[:], s_acc[:])
            nc.vector.tensor_mul(pm[:], s_acc[0:NCLS, :], ohi_sb[:])
        # row 4 of ssb is the total sum; DMA moves it to partition 0
        # (engines cannot read partition offsets that are not multiples of 32)
        nc.sync.dma_start(tot_sb[:], ssb[NCLS:NCLS + 1, :])

        # ---------------- tail ----------------
        with tc.tile_pool(name="pstail", bufs=1, space="PSUM") as pstail:
            p4 = pstail.tile([NCLS, RPC], F32)
            pos_ps = pstail.tile([1, RPC], F32)
            pmb = psm.tile([NCLS, RPC], BF16, tag="pmb")
            nc.vector.tensor_copy(pmb[:], pm[:])
            for it in range(RPC // 512):
                sl = slice(it * 512, (it + 1) * 512)
                nc.tensor.matmul(pos_ps[:, sl], ones_bf[0:NCLS, 0:1], pmb[:, sl],
                                 start=True, stop=True)
            # remove diag term; add eps to the denominator
            pos = psm.tile([1, RPC], F32, tag="pos")
            tot = psm.tile([1, RPC], F32, tag="tot")
            nc.vector.tensor_sub(pos[:], pos_ps[:], dexp[:])
            nc.vector.tensor_sub(tot[:], tot_sb[:], dexp[:])
            nc.vector.tensor_scalar_add(tot[:], tot[:], EPS)
            lt = psm.tile([1, RPC], F32, tag="lt")
            lp = psm.tile([1, RPC], F32, tag="lp")
            nc.scalar.activation(lt[:], tot[:], mybir.ActivationFunctionType.Ln)
            nc.scalar.activation(lp[:], pos[:], mybir.ActivationFunctionType.Ln)
            rl = psm.tile([1, RPC], BF16, tag="rl")
            nc.vector.tensor_sub(rl[:], lt[:], lp[:])
            # broadcast row_loss to 4 partitions, mask per class, reduce over i
            for it in range(RPC // 512):
                sl = slice(it * 512, (it + 1) * 512)
                nc.tensor.matmul(p4[:, sl], ones_row_bf[0:1, 0:NCLS], rl[:, sl],
                                 start=True, stop=True)
            cm = psm.tile([NCLS, RPC], F32, tag="cm")
            cls_f = psm.tile([NCLS, 1], F32, tag="clsf")
            nc.vector.tensor_mul(cm[:], p4[:], ohi_sb[:])
            nc.vector.reduce_sum(cls_f[:], cm[:], axis=mybir.AxisListType.X)
            nc.sync.dma_start(cls[:], cls_f[:])


def build():
    import os
    if not os.environ.get('NO_ACT_PATCH'):
        _patch_act_tables()
        _make_act_root()
    nc = bacc.Bacc("TRN2", target_bir_lowering=False, debug=False)
    et = nc.dram_tensor("et", [C, N], BF16, kind="ExternalInput").ap()
    rows = nc.dram_tensor("rows", [C, RPC], BF16, kind="ExternalInput").ap()
    ohj = nc.dram_tensor("ohj", [128, (NCLS + 1) * (N // 128)], BF16, kind="ExternalInput").ap()
    ohi = nc.dram_tensor("ohi", [NCLS, RPC], F32, kind="ExternalInput").ap()
    cls = nc.dram_tensor("cls", [NCLS, 1], F32, kind="ExternalOutput").ap()
    with tile.TileContext(nc) as tc:
        _build_body(tc, et, rows, ohj, ohi, cls)
    nc.compile()
    return nc


def _get_nc():
    if not _NC_CACHE:
        _NC_CACHE.append(build())
    return _NC_CACHE[0]


def make_in_maps(emb, labels):
    emb = np.asarray(emb, dtype=np.float32)
    lab = np.asarray(labels).astype(np.int64).reshape(-1)
    assert emb.shape == (2, C, 64, 64) and lab.shape == (N,)
    et = emb.reshape(2, C, N // 2)
    et = np.ascontiguousarray(
        np.concatenate([et[0], et[1]], axis=1)
    ).astype(ml_dtypes.bfloat16)  # [C, N]
    oh = (lab[:, None] == np.arange(NCLS)[None, :]).astype(np.float32)  # [N, 4]
    oh5 = np.concatenate([oh, np.ones((N, 1), np.float32)], axis=1)  # [N, 5]
    ohj = np.ascontiguousarray(
        oh5.reshape(N // 128, 128, NCLS + 1).transpose(1, 0, 2).reshape(128, -1)
    ).astype(ml_dtypes.bfloat16)
    in_maps = []
    for r in range(NCORES):
        sl = slice(r * RPC, (r + 1) * RPC)
        in_maps.append({
            "et": et,
            "rows": np.ascontiguousarray(et[:, sl]),
            "ohj": ohj,
            "ohi": np.ascontiguousarray(oh[sl].T),
        })
    return in_maps, lab


def finish(results, lab):
    sums = np.zeros(NCLS, np.float64)
    for r in results:
        sums += r["cls"].reshape(NCLS).astype(np.float64)
    counts = np.bincount(lab, minlength=NCLS).astype(np.float64)
    present = counts > 0
    per_cls = np.where(present, sums / np.maximum(counts, 1.0), 0.0)
    return np.float32(per_cls.sum() / present.sum())


def kernel(emb, labels):
    in_maps, lab = make_in_maps(emb, labels)
    nc = _get_nc()
    res = run_bass_kernel_spmd(nc, in_maps, core_ids=list(range(NCORES)))
    return finish(res.results, lab)
